# revision 14
# baseline (speedup 1.0000x reference)
"""ESIM-style bidirectional cross-attention (LocalInterface) Bass kernel for TRN2.

Full inputs: px [32,512,512] f32, hx [32,512,512] f32, p_mask/h_mask [32,512] bool.
Data-parallel over batch: 8 NeuronCores x 4 batches each. Returns (m_p, m_h),
each [32,512,2048] f32.

Per-batch math (per core, unrolled over 4 batches):
  e = px @ hx^T                          (PE, f32r: rounded at transpose-evict)
  u_bT[h,p] = exp(e^T - mb[p]) * hm[h]   built directly in [H,P] layout:
      e^T via PE transposes sharing one accumulation group per PSUM bank,
      -mb broadcast on top via K=1 ones-matmuls, unbiased row max for
      stability, per-partition 0/1 mask multiply zeroes masked h rows.
  px_hat = (u_bT^T @ hx) / s_b           (PE bf16; s_b via ones-column matmuls)
  symmetric for hx_hat via u_aT[p,h] = exp(e - ma[h]) * pm[p].
  m_p rows assemble in one [128, 4x512] SBUF tile (px DMA'd into segment 0,
  px_hat/diff/prod written in place) so each output block is a single 1MB DMA
  with 8KB contiguous runs.
"""

import numpy as np

NB = 4          # batches per core
NCORES = 8
S = 512         # P = H = D = 512
NBLK = 4        # 512 / 128

_CACHED = {}


def _build():
    import concourse.tile as tile
    import concourse.mybir as mybir
    from concourse import bacc
    from concourse.masks import make_identity

    F32 = mybir.dt.float32
    F32R = mybir.dt.float32r
    BF16 = mybir.dt.bfloat16
    EXP = mybir.ActivationFunctionType.Exp
    COPY = mybir.ActivationFunctionType.Copy
    AX = mybir.AxisListType.X

    nc = bacc.Bacc(None, target_bir_lowering=False)
    px_d = nc.dram_tensor("px", [NB, S, S], F32, kind="ExternalInput")
    hx_d = nc.dram_tensor("hx", [NB, S, S], F32, kind="ExternalInput")
    hm_d = nc.dram_tensor("hm", [NB, S], F32, kind="ExternalInput")  # 1.0 keep, 0.0 masked
    pm_d = nc.dram_tensor("pm", [NB, S], F32, kind="ExternalInput")
    mp_d = nc.dram_tensor("mp", [NB, S, 4 * S], F32, kind="ExternalOutput")
    mh_d = nc.dram_tensor("mh", [NB, S, 4 * S], F32, kind="ExternalOutput")

    with tile.TileContext(nc) as tc:
        with (
            tc.tile_pool(name="const", bufs=1) as const,
            tc.tile_pool(name="sbA", bufs=2) as sbA,
            tc.tile_pool(name="sbL", bufs=3) as sbL,
            tc.tile_pool(name="sbT", bufs=8) as sbT,
            tc.tile_pool(name="sbB", bufs=4) as sbB,
            tc.tile_pool(name="sbS", bufs=2) as sbS,
            tc.tile_pool(name="sbO", bufs=3) as sbO,
            tc.tile_pool(name="ppin", bufs=2, space="PSUM") as ppin,
            tc.tile_pool(name="pepb", bufs=2, space="PSUM") as pepb,
            tc.tile_pool(name="ppet", bufs=2, space="PSUM") as ppet,
            tc.tile_pool(name="pval", bufs=2, space="PSUM") as pval,
        ):
            ident = const.tile([128, 128], F32)
            make_identity(nc, ident)
            identb = const.tile([128, 128], BF16)
            nc.vector.tensor_copy(out=identb, in_=ident)
            ones_row = const.tile([1, 128], BF16)
            nc.vector.memset(ones_row, 1.0)
            ones_col = const.tile([128, 1], BF16)
            nc.vector.memset(ones_col, 1.0)
            # all batches' keep-masks in one small load: [:, b, j] per-partition cols
            hmask = const.tile([128, NB, NBLK], F32)
            pmask = const.tile([128, NB, NBLK], F32)
            nc.sync.dma_start(out=hmask, in_=hm_d.rearrange("b (j r) -> r b j", r=128))
            nc.sync.dma_start(out=pmask, in_=pm_d.rearrange("b (j r) -> r b j", r=128))

            for b in range(NB):
                # ---- load ----
                px_t = sbL.tile([128, NBLK, S], F32, tag="px_t")
                hx_t = sbL.tile([128, NBLK, S], F32, tag="hx_t")
                nc.sync.dma_start(out=px_t, in_=px_d[b].rearrange("(i r) d -> r i d", r=128))
                nc.sync.dma_start(out=hx_t, in_=hx_d[b].rearrange("(i r) d -> r i d", r=128))

                # bf16 copies for value-matmul rhs
                px_b = sbA.tile([128, NBLK, S], BF16, tag="px_b")
                hx_b = sbA.tile([128, NBLK, S], BF16, tag="hx_b")
                for i in range(NBLK):
                    nc.vector.tensor_copy(out=px_b[:, i], in_=px_t[:, i])
                    nc.vector.tensor_copy(out=hx_b[:, i], in_=hx_t[:, i])

                # ---- input transposes: pxT[d,p], hxT[d,h] (f32r rounded on evict) ----
                pxTr = [sbT.tile([128, S], F32R, tag="pxTr", name=f"pxTr{b}_{j}") for j in range(NBLK)]
                hxTr = [sbT.tile([128, S], F32R, tag="hxTr", name=f"hxTr{b}_{j}") for j in range(NBLK)]
                for src, dst in ((px_t, pxTr), (hx_t, hxTr)):
                    for j in range(NBLK):
                        pin = ppin.tile([128, S], F32, tag="pin")
                        for i in range(NBLK):
                            nc.tensor.transpose(
                                pin[:, 128 * i:128 * (i + 1)],
                                src[:, i, 128 * j:128 * (j + 1)],
                                ident,
                            )
                        nc.scalar.copy(out=dst[j], in_=pin)

                # ---- e = px @ hx^T  [P,H], f32r ----
                e_sb = [sbT.tile([128, S], F32, tag="e_sb", name=f"e_sb{b}_{i}") for i in range(NBLK)]
                negm_b = sbS.tile([128, NBLK], F32, tag="negm_b")
                for i in range(NBLK):
                    pe = pepb.tile([128, S], F32, tag="pe")
                    for j in range(NBLK):
                        nc.tensor.matmul(
                            pe, pxTr[j][:, 128 * i:128 * (i + 1)], hxTr[j],
                            start=(j == 0), stop=(j == NBLK - 1),
                        )
                    nc.scalar.copy(out=e_sb[i], in_=pe)
                    nc.vector.reduce_max(
                        out=negm_b[:, i:i + 1], in_=e_sb[i], axis=AX, negate=True
                    )

                # ---- stat rows for dir b: -mb as 4 x [1,128] bf16 rows ----
                negmb_bf = sbS.tile([128, NBLK], BF16, tag="negmb_bf")
                nc.vector.tensor_copy(out=negmb_bf, in_=negm_b)
                rows_b = sbS.tile([1, NBLK, 128], BF16, tag="rows_b")
                for i in range(NBLK):
                    rp = ppin.tile([1, 128], BF16, tag="pin")
                    nc.tensor.transpose(rp, negmb_bf[:, i:i + 1], identb)
                    nc.scalar.copy(out=rows_b[:, i], in_=rp)

                # ---- eT stream: transpose e, read ma, add -mb bcast, exp -> u_bT ----
                u_bT = sbA.tile([128, NBLK, S], BF16, tag="u_bT")
                negm_a = sbS.tile([128, NBLK], F32, tag="negm_a")
                for j in range(NBLK):
                    pet = ppet.tile([128, S], F32, tag="pet")
                    for i in range(NBLK):
                        # one accumulation group for the whole bank: first
                        # transpose claims the 2KB zero-region, the rest
                        # overwrite their own pending-zero slices, and the
                        # -mb broadcast matmuls below then accumulate on top.
                        nc.tensor.matmul(
                            pet[:, 128 * i:128 * (i + 1)],
                            e_sb[i][:, 128 * j:128 * (j + 1)],
                            ident,
                            is_transpose=True,
                            start=(i == 0), stop=False, skip_group_check=True,
                        )
                    nc.vector.reduce_max(
                        out=negm_a[:, j:j + 1], in_=pet, axis=AX, negate=True
                    )
                    for i in range(NBLK):
                        nc.tensor.matmul(
                            pet[:, 128 * i:128 * (i + 1)], ones_row, rows_b[:, i],
                            start=False, stop=(i == NBLK - 1), skip_group_check=True,
                        )
                    nc.scalar.activation(out=u_bT[:, j], in_=pet, func=EXP)
                    nc.vector.tensor_scalar_mul(
                        out=u_bT[:, j], in0=u_bT[:, j], scalar1=hmask[:, b, j:j + 1]
                    )

                # ---- stat rows for dir a ----
                negma_bf = sbS.tile([128, NBLK], BF16, tag="negma_bf")
                nc.vector.tensor_copy(out=negma_bf, in_=negm_a)
                rows_a = sbS.tile([1, NBLK, 128], BF16, tag="rows_a")
                for j in range(NBLK):
                    rp = ppin.tile([1, 128], BF16, tag="pin")
                    nc.tensor.transpose(rp, negma_bf[:, j:j + 1], identb)
                    nc.scalar.copy(out=rows_a[:, j], in_=rp)

                # ---- -ma broadcast tile [128p, 512h] ----
                pbc = pepb.tile([128, S], F32, tag="pe")
                for j in range(NBLK):
                    nc.tensor.matmul(
                        pbc[:, 128 * j:128 * (j + 1)], ones_row, rows_a[:, j],
                        start=True, stop=True, skip_group_check=True,
                    )

                # ---- u_aT[p,h] = exp(e - ma) * pm ----
                u_aT = sbA.tile([128, NBLK, S], BF16, tag="u_aT")
                for i in range(NBLK):
                    tmp = sbB.tile([128, S], F32, tag="tmp_a")
                    nc.vector.tensor_add(tmp, e_sb[i], pbc)
                    nc.scalar.activation(out=u_aT[:, i], in_=tmp, func=EXP)
                    nc.vector.tensor_scalar_mul(
                        out=u_aT[:, i], in0=u_aT[:, i], scalar1=pmask[:, b, i:i + 1]
                    )

                # ---- value matmuls + s + outputs, direction b (px_hat, m_p) ----
                s_ps = pepb.tile([128, 2 * NBLK], F32, tag="pe")
                r_t = sbS.tile([128, 2 * NBLK], F32, tag="r_t")
                for i in range(NBLK):
                    pv = pval.tile([128, S], F32, tag="pv")
                    for j in range(NBLK):
                        nc.tensor.matmul(
                            pv, u_bT[:, j, 128 * i:128 * (i + 1)], hx_b[:, j],
                            start=(j == 0), stop=(j == NBLK - 1),
                        )
                        nc.tensor.matmul(
                            s_ps[:, i:i + 1], u_bT[:, j, 128 * i:128 * (i + 1)], ones_col,
                            start=(j == 0), stop=(j == NBLK - 1), skip_group_check=True,
                        )
                    nc.vector.reciprocal(out=r_t[:, i:i + 1], in_=s_ps[:, i:i + 1])
                    mpb = sbO.tile([128, NBLK, S], F32, tag="mp_blk")
                    nc.gpsimd.tensor_copy(out=mpb[:, 0], in_=px_t[:, i])
                    nc.scalar.activation(
                        out=mpb[:, 1], in_=pv, func=COPY, scale=r_t[:, i:i + 1]
                    )
                    nc.vector.tensor_sub(mpb[:, 2], px_t[:, i], mpb[:, 1])
                    nc.gpsimd.tensor_mul(mpb[:, 3], px_t[:, i], mpb[:, 1])
                    nc.sync.dma_start(
                        out=mp_d[b, 128 * i:128 * (i + 1), :],
                        in_=mpb.rearrange("r f s -> r (f s)"),
                    )

                # ---- direction a (hx_hat, m_h) ----
                for j in range(NBLK):
                    pv = pval.tile([128, S], F32, tag="pv")
                    for i in range(NBLK):
                        nc.tensor.matmul(
                            pv, u_aT[:, i, 128 * j:128 * (j + 1)], px_b[:, i],
                            start=(i == 0), stop=(i == NBLK - 1),
                        )
                        nc.tensor.matmul(
                            s_ps[:, NBLK + j:NBLK + j + 1],
                            u_aT[:, i, 128 * j:128 * (j + 1)], ones_col,
                            start=(i == 0), stop=(i == NBLK - 1), skip_group_check=True,
                        )
                    nc.vector.reciprocal(
                        out=r_t[:, NBLK + j:NBLK + j + 1],
                        in_=s_ps[:, NBLK + j:NBLK + j + 1],
                    )
                    mhb = sbO.tile([128, NBLK, S], F32, tag="mh_blk")
                    nc.gpsimd.tensor_copy(out=mhb[:, 0], in_=hx_t[:, j])
                    nc.scalar.activation(
                        out=mhb[:, 1], in_=pv, func=COPY,
                        scale=r_t[:, NBLK + j:NBLK + j + 1],
                    )
                    nc.vector.tensor_sub(mhb[:, 2], hx_t[:, j], mhb[:, 1])
                    nc.gpsimd.tensor_mul(mhb[:, 3], hx_t[:, j], mhb[:, 1])
                    nc.sync.dma_start(
                        out=mh_d[b, 128 * j:128 * (j + 1), :],
                        in_=mhb.rearrange("r f s -> r (f s)"),
                    )

    nc.compile()
    return nc


def _get_nc():
    if "nc" not in _CACHED:
        _CACHED["nc"] = _build()
    return _CACHED["nc"]


def run_sharded(px, hx, p_mask, h_mask, **kw):
    """Shard over batch, run on 8 cores, return (results, BassKernelResults)."""
    from concourse.bass_utils import run_bass_kernel_spmd

    nc = _get_nc()
    hm = (~np.asarray(h_mask)).astype(np.float32)
    pm = (~np.asarray(p_mask)).astype(np.float32)
    in_maps = []
    for c in range(NCORES):
        sl = slice(NB * c, NB * (c + 1))
        in_maps.append({
            "px": np.ascontiguousarray(px[sl], dtype=np.float32),
            "hx": np.ascontiguousarray(hx[sl], dtype=np.float32),
            "hm": np.ascontiguousarray(hm[sl]),
            "pm": np.ascontiguousarray(pm[sl]),
        })
    res = run_bass_kernel_spmd(nc, in_maps, core_ids=list(range(NCORES)), **kw)
    mp = np.concatenate([res.results[c]["mp"] for c in range(NCORES)], axis=0)
    mh = np.concatenate([res.results[c]["mh"] for c in range(NCORES)], axis=0)
    return (mp, mh), res


def kernel(px, hx, p_mask, h_mask):
    (mp, mh), _ = run_sharded(px, hx, p_mask, h_mask)
    return mp, mh
